# revision 1
# baseline (speedup 1.0000x reference)
"""Trainium2 Bass kernel for nn_DiagMean (histogram_binning).

Computes, per batch image A [T, T]: the mean over each diagonal d = j - i
(for |d| <= T/2, excluding the last element of each diagonal), then centers
across the T+1 diagonal bins and negates.

Strategy (pure data-parallel over batch, 2 images per core on 8 cores):
  - Skewed DMA: for a 128-row tile starting at row i0, the access pattern
    [partition stride T+1, free stride 1] reads S[p, k] = A[i0+p, i0+p+k-T/2],
    so diagonal bin k is a *column* of S. Each partition's read is contiguous
    in HBM, so DMA runs at full bandwidth.
  - Columns fully outside any valid diagonal are never loaded (per-tile
    [kmin, kmax] window): ~22% HBM traffic saved.
  - The trapezoid validity boundary reduces to two shared 128x128 masks
    (prefix: p+m >= 128, suffix: p+m <= 126) applied to one 128-wide column
    strip per side per tile.
  - Column sums via TensorE: ones[128,1].T @ S chunk -> PSUM [1, N],
    accumulated over the 16 row-tiles of an image (fp32r, 1 cycle/col).

Scheduling:
  - ones/zrhs const DMAs go FIRST on the gpsimd queue so TensorE's zeroing
    matmuls (and with them the DMA queues' sem-reuse windows, which are
    keyed to matmul progress) start ~20us earlier.
  - The sync/scalar (HWDGE) instruction streams carry ONLY tile-load
    dma_starts mid-kernel: the whole epilogue is DVE (PSUM-read multiply,
    per-chunk reduce_sum, centering), and image 0's output store rides the
    gpsimd queue, so no epilogue op ever head-of-line blocks a DMA issue.
  - PSUM bank plan: each of the 5 accumulation groups per image (4 column
    chunks + the k=2048 column) has its own bank; image 1 reuses only banks
    image 0 frees early (c3/c4 are read mid-loop, one tile after their last
    accumulation), so image 1's matmuls never wait on image 0's end.
"""

import numpy as np

import concourse.bacc as bacc
import concourse.bass as bass
import concourse.mybir as mybir
import concourse.tile as tile
from concourse.bass_utils import run_bass_kernel_spmd

B, T = 16, 2048
HALF = T // 2          # 1024
K = T + 1              # 2049 diagonal bins
N_CORES = 8
BPC = B // N_CORES     # images per core
P = 128
NT = T // P            # row tiles per image
BANK = 512             # fp32 elements per PSUM bank

_nc_cache = None


def _build():
    nc = bacc.Bacc("TRN2", target_bir_lowering=False, debug=False)
    f32 = mybir.dt.float32
    f32r = mybir.dt.float32r

    # Front-padded with HALF zeros so tile 0's skewed read of row 0 lands in
    # the pad instead of out of bounds (the pad zeros are exactly the masked
    # region, so no special-casing).
    x = nc.dram_tensor("x", [HALF + BPC * T * T], f32, kind="ExternalInput")
    y = nc.dram_tensor("y", [BPC, K], f32, kind="ExternalOutput")

    pp_ = np.arange(P)[:, None]
    mm_ = np.arange(P)[None, :]
    maskP_np = (pp_ + mm_ >= P).astype(np.float32)        # prefix validity
    maskS_np = (pp_ + mm_ <= P - 2).astype(np.float32)    # suffix validity
    counts = (T - 1 - np.abs(np.arange(-HALF, HALF + 1))).astype(np.float64)
    # Pad row length to even so SBUF partition strides stay 8-byte aligned.
    KP = K + 1
    negrecip_np = np.zeros((1, KP), dtype=np.float32)
    negrecip_np[0, :K] = (-1.0 / counts).astype(np.float32)
    ones_np = np.ones((P, 1), dtype=np.float32)
    zrhs_np = np.zeros((2, BANK), dtype=np.float32)

    maskP_d = nc.inline_tensor(maskP_np, name="maskP")
    maskS_d = nc.inline_tensor(maskS_np, name="maskS")
    negrecip_d = nc.inline_tensor(negrecip_np, name="negrecip")
    ones_d = nc.inline_tensor(ones_np, name="onesw")
    zrhs_d = nc.inline_tensor(zrhs_np, name="zrhs")

    # Per-tile valid column windows.
    tinfo = []
    for t in range(NT):
        i0 = t * P
        kmin = max(0, 896 - i0)
        kmax = min(T, 3070 - i0)
        tinfo.append((i0, kmin, kmax))
    NCH = T // BANK                                     # 4 (columns 0..2047)
    last = {}
    for t, (i0, kmin, kmax) in enumerate(tinfo):
        for c in range(NCH):
            lo, hi = c * BANK, c * BANK + BANK - 1
            if kmin <= hi and min(kmax, T - 1) >= lo:
                last[c] = t
        if kmax == T:  # column k=2048 (diag +1024) -> group 4
            last[4] = t

    # PSUM bank plan: 8 banks, 5 groups per image. Image 1's groups reuse
    # banks image 0 frees early (c3 read after its tile 12, c4 after tile 8).
    BANK_TAG = [
        {0: "bk0", 1: "bk1", 2: "bk2", 3: "bk3", 4: "bk7"},
        {0: "bk3", 1: "bk4", 2: "bk5", 3: "bk6", 4: "bk7"},
    ]

    with tile.TileContext(nc) as tc:
        with (
            tc.tile_pool(name="consts", bufs=1) as consts,
            tc.tile_pool(name="data", bufs=18) as data,
            tc.tile_pool(name="acc", bufs=1, space=bass.MemorySpace.PSUM) as accp,
            tc.tile_pool(name="post", bufs=1) as post,
        ):
            maskP = consts.tile([P, P], f32r)
            maskS = consts.tile([P, P], f32r)
            negrecip = consts.tile([1, KP], f32)
            ones = consts.tile([P, 1], f32r)
            zrhs = consts.tile([2, BANK], f32r)
            # ones/zrhs first: TensorE's zeroing matmuls depend only on these
            # two tiny transfers, so they clear the gpsimd queue ~20us before
            # the baseline ordering did.
            nc.gpsimd.dma_start(ones[:], ones_d[:].bitcast(f32r))
            nc.gpsimd.dma_start(zrhs[:], zrhs_d[:].bitcast(f32r))
            nc.gpsimd.dma_start(maskP[:], maskP_d[:].bitcast(f32r))
            nc.gpsimd.dma_start(maskS[:], maskS_d[:].bitcast(f32r))
            nc.gpsimd.dma_start(negrecip[:], negrecip_d[:])
            # Collector: touch each const on VectorE once so later DVE ops
            # inherit the const-DMA sync via engine program order instead of
            # each carrying its own semaphore wait (ISA wait-slot limit).
            warm = consts.tile([P, P], f32)
            nc.vector.tensor_copy(warm[:], maskP[:].bitcast(f32))
            nc.vector.tensor_copy(warm[:], maskS[:].bitcast(f32))
            warm2 = consts.tile([1, KP], f32)
            nc.vector.tensor_copy(warm2[:], negrecip[:])

            nm_t = [post.tile([1, KP], f32, tag=f"nm{i}", name=f"nm{i}") for i in range(BPC)]
            s1_t = [post.tile([1, 1], f32, tag=f"s1{i}", name=f"s1{i}") for i in range(BPC)]
            av_t = [post.tile([1, 1], f32, tag=f"av{i}", name=f"av{i}") for i in range(BPC)]
            ot_t = [post.tile([1, KP], f32, tag=f"ot{i}", name=f"ot{i}") for i in range(BPC)]
            pp_t = [post.tile([1, 6], f32, tag=f"pp{i}", name=f"pp{i}") for i in range(BPC)]

            def ep_tail(b, pt):
                # Last three chunk reads, combine, centering, output store.
                # For image 0 this is issued a few tiles INTO image 1's loop,
                # so it sits after image 1's first masks in the DVE stream:
                # the scalar queue's sem-reuse waits (keyed to the DVE op
                # counter) stop stalling image 1's tile loads behind it.
                nm, s1, avgneg, ot = nm_t[b], s1_t[b], av_t[b], ot_t[b]
                for g in (2, 1, 0):
                    ep_group(b, g, pt)
                nc.vector.reduce_sum(
                    s1[0:1, 0:1], pp_t[b][0:1, 0:5], axis=mybir.AxisListType.X
                )
                nc.vector.tensor_scalar_mul(
                    avgneg[0:1, 0:1], s1[0:1, 0:1], -1.0 / K
                )
                nc.vector.tensor_scalar_add(
                    ot[0:1, 0:K], nm[0:1, 0:K], avgneg[0:1, 0:1]
                )
                if b == 0:
                    # gpsimd queue is idle mid-kernel; keeps the sync queue's
                    # image-1 tile loads unblocked.
                    nc.gpsimd.dma_start(y[b : b + 1, :], ot[0:1, 0:K])
                else:
                    nc.sync.dma_start(y[b : b + 1, :], ot[0:1, 0:K])

            def ep_group(b, g, pt):
                # negmean for group g (the DVE read frees its PSUM bank),
                # then a DVE reduce accumulates the chunk sum into pp[g].
                # All-DVE so the scalar (ACT) engine stays a pure DMA queue.
                nm, pp = nm_t[b], pp_t[b]
                if g == 4:
                    nc.vector.tensor_mul(
                        nm[0:1, T : T + 1],
                        pt[4][0:1, 1:2],
                        negrecip[:, T : T + 1],
                    )
                    nc.vector.tensor_copy(pp[0:1, 4:5], nm[0:1, T : T + 1])
                    return
                lo = g * BANK
                nc.vector.tensor_mul(
                    nm[0:1, lo : lo + BANK],
                    pt[g][0:1, 0:BANK],
                    negrecip[:, lo : lo + BANK],
                )
                nc.vector.reduce_sum(
                    pp[0:1, g : g + 1],
                    nm[0:1, lo : lo + BANK],
                    axis=mybir.AxisListType.X,
                )

            for b in range(BPC):
                base = b * T * T
                nm, s1, avgneg, ot = nm_t[b], s1_t[b], av_t[b], ot_t[b]
                pt = {
                    g: accp.tile(
                        [1, BANK], f32, tag=BANK_TAG[b][g], name=f"p{b}g{g}"
                    )
                    for g in range(5)
                }

                def chunk_dst(c, lo, hi, pt=pt):
                    return pt[c][0:1, lo - c * BANK : hi + 1 - c * BANK]

                # A start=True matmul arms zeroing for its whole 2KB PSUM
                # bank, so every accumulation group begins with one full-bank
                # zeroing matmul; all data matmuls then accumulate.
                for g in range(5):
                    nc.tensor.matmul(
                        pt[g][0:1, 0:BANK],
                        ones[0:2, :],
                        zrhs[:],
                        start=True,
                        stop=False,
                    )

                for t, (i0, kmin, kmax) in enumerate(tinfo):
                    rows = P - 1 if t == NT - 1 else P
                    # fp32r matmul needs even N; suffix-capped kmax is even, so
                    # load one extra column there (maskS's last column zeroes it).
                    kmax_dma = kmax + 1 if i0 >= 1024 else kmax
                    width = kmax_dma - kmin + 1
                    S = data.tile([P, KP], f32r, tag="S")
                    dma_eng = nc.sync if (b * NT + t) % 2 == 0 else nc.scalar
                    dma_eng.dma_start(
                        S[0:rows, kmin : kmax_dma + 1],
                        bass.AP(
                            x,
                            base + i0 * (T + 1) + kmin,
                            [[T + 1, rows], [1, width]],
                        ).bitcast(f32r),
                    )
                    if i0 <= 896:
                        a = 896 - i0
                        nc.vector.tensor_mul(
                            S[0:rows, a : a + P],
                            S[0:rows, a : a + P],
                            maskP[0:rows, :],
                        )
                    if i0 >= 896:
                        a = 2944 - i0
                        w = kmax_dma + 1 - a
                        nc.vector.tensor_mul(
                            S[0:rows, a : a + w],
                            S[0:rows, a : a + w],
                            maskS[0:rows, 0:w],
                        )
                    for c in range(NCH):
                        lo = max(kmin, c * BANK)
                        hi = min(min(kmax, T - 1), c * BANK + BANK - 1)
                        if lo > hi:
                            continue
                        if (hi - lo + 1) % 2 == 1:
                            hi += 1
                            assert hi <= min(kmax_dma, c * BANK + BANK - 1)
                        nc.tensor.matmul(
                            chunk_dst(c, lo, hi),
                            ones[0:rows, :],
                            S[0:rows, lo : hi + 1],
                            start=False,
                            stop=(last[c] == t),
                        )
                    if kmax == T:
                        # fp32r matmul needs N>=2: recompute col 2047 into a
                        # scratch lane and keep only col 2048's sum.
                        nc.tensor.matmul(
                            pt[4][0:1, 0:2],
                            ones[0:rows, :],
                            S[0:rows, T - 1 : T + 1],
                            start=False,
                            stop=(last[4] == t),
                        )
                    # Interleaved epilogue: read each accumulation group one
                    # tile after it closes, so its bank frees mid-loop and
                    # the next image's matmuls never wait on this image's end.
                    if t == last[4] + 1:
                        ep_group(b, 4, pt)
                    if t == last[3] + 1:
                        ep_group(b, 3, pt)
                    if b == 1 and t == 2:
                        ep_tail(0, tail_pt)

                if b == 0:
                    tail_pt = pt
                else:
                    ep_tail(1, pt)
    nc.compile()
    return nc


def kernel(**inputs: np.ndarray) -> np.ndarray:
    global _nc_cache
    x = np.asarray(inputs["inputs"], dtype=np.float32)
    assert x.shape == (B, T, T)
    if _nc_cache is None:
        _nc_cache = _build()
    pad = np.zeros(HALF, dtype=np.float32)
    in_maps = [
        {
            "x": np.concatenate(
                [pad, np.ascontiguousarray(x[c * BPC : (c + 1) * BPC]).reshape(-1)]
            )
        }
        for c in range(N_CORES)
    ]
    res = run_bass_kernel_spmd(_nc_cache, in_maps, core_ids=list(range(N_CORES)))
    return np.concatenate([r["y"] for r in res.results], axis=0)



# revision 2
# speedup vs baseline: 1.1159x; 1.1159x over previous
"""Trainium2 Bass kernel for nn_DiagMean (histogram_binning).

Computes, per batch image A [T, T]: the mean over each diagonal d = j - i
(for |d| <= T/2, excluding the last element of each diagonal), then centers
across the T+1 diagonal bins and negates.

Strategy (pure data-parallel over batch, 2 images per core on 8 cores):
  - Skewed DMA: for a 128-row tile starting at row i0, the access pattern
    [partition stride T+1, free stride 1] reads S[p, k] = A[i0+p, i0+p+k-T/2],
    so diagonal bin k is a *column* of S. Each partition's read is contiguous
    in HBM, so DMA runs at full per-descriptor rate.
  - PAIRED loads: two consecutive 128-row tiles are fetched by ONE dma_start
    via a 3D access pattern (partition, section, column) into a [128, 2*WU]
    tile, WU = the pair's union column window. Halves the number of DMA
    instructions and semaphore waits; each issue queues ~1.6-2.1 MB of
    descriptors, so the 16 SDMA engines stay fed. Union windows cost ~4%
    extra HBM traffic vs exact per-tile windows.
  - Loads are spread round-robin over THREE issue queues (sync, scalar =
    HWDGE rings; gpsimd = SWDGE) so no single sequencer's sem-reuse window
    (8 DMAHW + 8 DMASW lanes) stalls descriptor supply.
  - Columns fully outside any valid diagonal are never loaded (per-pair
    [kminU, kmaxU] window): ~22% HBM traffic saved vs full rows.
  - The trapezoid validity boundary reduces to two shared 128x128 masks
    (prefix: p+m >= 128, suffix: p+m <= 126) applied to one 128-wide column
    strip per side per tile.
  - Column sums via TensorE: ones[128,1].T @ S chunk -> PSUM [1, N],
    accumulated over the 16 row-tiles of an image (fp32r, 1 cycle/col).

Scheduling:
  - ones/zrhs const DMAs go FIRST on the gpsimd queue so TensorE's zeroing
    matmuls start early.
  - The epilogue is all-DVE (PSUM-read multiply, per-chunk reduce_sum,
    centering); image 0's output store rides the gpsimd queue so it never
    head-of-line blocks a tile-load issue.
  - PSUM bank plan: each of the 5 accumulation groups per image (4 column
    chunks + the k=2048 column) has its own bank; image 1 reuses only banks
    image 0 frees early (c3/c4 are read mid-loop, one tile after their last
    accumulation), so image 1's matmuls never wait on image 0's end.
"""

import numpy as np

import concourse.bacc as bacc
import concourse.bass as bass
import concourse.mybir as mybir
import concourse.tile as tile
from concourse.bass_utils import run_bass_kernel_spmd

B, T = 16, 2048
HALF = T // 2          # 1024
K = T + 1              # 2049 diagonal bins
N_CORES = 8
BPC = B // N_CORES     # images per core
P = 128
NT = T // P            # row tiles per image
BANK = 512             # fp32 elements per PSUM bank
TAILPAD = 2560         # rear pad: pair-DMAs always read 128 rows/section

_nc_cache = None


def _build():
    nc = bacc.Bacc("TRN2", target_bir_lowering=False, debug=False)
    f32 = mybir.dt.float32
    f32r = mybir.dt.float32r

    # Front-padded with HALF zeros so tile 0's skewed read of row 0 lands in
    # the pad instead of out of bounds (the pad zeros are exactly the masked
    # region, so no special-casing). Rear pad covers the last pair's
    # always-128-row section read past the image end (rows there are sliced
    # out of every matmul).
    x = nc.dram_tensor("x", [HALF + BPC * T * T + TAILPAD], f32, kind="ExternalInput")
    y = nc.dram_tensor("y", [BPC, K], f32, kind="ExternalOutput")

    pp_ = np.arange(P)[:, None]
    mm_ = np.arange(P)[None, :]
    maskP_np = (pp_ + mm_ >= P).astype(np.float32)        # prefix validity
    maskS_np = (pp_ + mm_ <= P - 2).astype(np.float32)    # suffix validity
    counts = (T - 1 - np.abs(np.arange(-HALF, HALF + 1))).astype(np.float64)
    # Pad row length to even so SBUF partition strides stay 8-byte aligned.
    KP = K + 1
    negrecip_np = np.zeros((1, KP), dtype=np.float32)
    negrecip_np[0, :K] = (-1.0 / counts).astype(np.float32)
    ones_np = np.ones((P, 1), dtype=np.float32)
    zrhs_np = np.zeros((2, BANK), dtype=np.float32)

    maskP_d = nc.inline_tensor(maskP_np, name="maskP")
    maskS_d = nc.inline_tensor(maskS_np, name="maskS")
    negrecip_d = nc.inline_tensor(negrecip_np, name="negrecip")
    ones_d = nc.inline_tensor(ones_np, name="onesw")
    zrhs_d = nc.inline_tensor(zrhs_np, name="zrhs")

    # Per-tile valid column windows.
    tinfo = []
    for t in range(NT):
        i0 = t * P
        kmin = max(0, 896 - i0)
        kmax = min(T, 3070 - i0)
        kd = kmax + 1 if i0 >= 1024 else kmax  # even-N spare column
        tinfo.append((i0, kmin, kmax, kd))
    NCH = T // BANK                                     # 4 (columns 0..2047)
    last = {}
    for t, (i0, kmin, kmax, kd) in enumerate(tinfo):
        for c in range(NCH):
            lo, hi = c * BANK, c * BANK + BANK - 1
            if kmin <= hi and min(kmax, T - 1) >= lo:
                last[c] = t
        if kmax == T:  # column k=2048 (diag +1024) -> group 4
            last[4] = t

    # Pair (2-tile) DMA windows: union of the two tiles' column windows,
    # width padded even (the pad column is loaded but never consumed).
    pinfo = []
    for q in range(NT // 2):
        t0, t1 = 2 * q, 2 * q + 1
        kminU = min(tinfo[t0][1], tinfo[t1][1])
        kmaxU = max(tinfo[t0][3], tinfo[t1][3])
        WU = kmaxU - kminU + 1
        if WU % 2 == 1:
            WU += 1
        pinfo.append((kminU, WU))

    # PSUM bank plan: 8 banks, 5 groups per image. Image 1's groups reuse
    # banks image 0 frees early (c3 read after its tile 12, c4 after tile 8).
    BANK_TAG = [
        {0: "bk0", 1: "bk1", 2: "bk2", 3: "bk3", 4: "bk7"},
        {0: "bk3", 1: "bk4", 2: "bk5", 3: "bk6", 4: "bk7"},
    ]

    with tile.TileContext(nc) as tc:
        with (
            tc.tile_pool(name="consts", bufs=1) as consts,
            tc.tile_pool(name="data", bufs=8) as data,
            tc.tile_pool(name="acc", bufs=1, space=bass.MemorySpace.PSUM) as accp,
            tc.tile_pool(name="post", bufs=1) as post,
        ):
            maskP = consts.tile([P, P], f32r)
            maskS = consts.tile([P, P], f32r)
            negrecip = consts.tile([1, KP], f32)
            ones = consts.tile([P, 1], f32r)
            zrhs = consts.tile([2, BANK], f32r)
            # ones/zrhs first: TensorE's zeroing matmuls depend only on these
            # two tiny transfers.
            nc.gpsimd.dma_start(ones[:], ones_d[:].bitcast(f32r))
            nc.gpsimd.dma_start(zrhs[:], zrhs_d[:].bitcast(f32r))
            nc.gpsimd.dma_start(maskP[:], maskP_d[:].bitcast(f32r))
            nc.gpsimd.dma_start(maskS[:], maskS_d[:].bitcast(f32r))
            nc.gpsimd.dma_start(negrecip[:], negrecip_d[:])
            # Collector: touch each const on VectorE once so later DVE ops
            # inherit the const-DMA sync via engine program order instead of
            # each carrying its own semaphore wait (ISA wait-slot limit).
            warm = consts.tile([P, P], f32)
            nc.vector.tensor_copy(warm[:], maskP[:].bitcast(f32))
            nc.vector.tensor_copy(warm[:], maskS[:].bitcast(f32))
            warm2 = consts.tile([1, KP], f32)
            nc.vector.tensor_copy(warm2[:], negrecip[:])

            nm_t = [post.tile([1, KP], f32, tag=f"nm{i}", name=f"nm{i}") for i in range(BPC)]
            s1_t = [post.tile([1, 1], f32, tag=f"s1{i}", name=f"s1{i}") for i in range(BPC)]
            av_t = [post.tile([1, 1], f32, tag=f"av{i}", name=f"av{i}") for i in range(BPC)]
            ot_t = [post.tile([1, KP], f32, tag=f"ot{i}", name=f"ot{i}") for i in range(BPC)]
            pp_t = [post.tile([1, 6], f32, tag=f"pp{i}", name=f"pp{i}") for i in range(BPC)]

            def ep_tail(b, pt):
                # Last three chunk reads, combine, centering, output store.
                # For image 0 this is issued a few tiles INTO image 1's loop.
                nm, s1, avgneg, ot = nm_t[b], s1_t[b], av_t[b], ot_t[b]
                for g in (2, 1, 0):
                    ep_group(b, g, pt)
                nc.vector.reduce_sum(
                    s1[0:1, 0:1], pp_t[b][0:1, 0:5], axis=mybir.AxisListType.X
                )
                nc.vector.tensor_scalar_mul(
                    avgneg[0:1, 0:1], s1[0:1, 0:1], -1.0 / K
                )
                nc.vector.tensor_scalar_add(
                    ot[0:1, 0:K], nm[0:1, 0:K], avgneg[0:1, 0:1]
                )
                if b == 0:
                    # keeps the HWDGE queues' tile loads unblocked.
                    nc.gpsimd.dma_start(y[b : b + 1, :], ot[0:1, 0:K])
                else:
                    nc.sync.dma_start(y[b : b + 1, :], ot[0:1, 0:K])

            def ep_group(b, g, pt):
                # negmean for group g (the DVE read frees its PSUM bank),
                # then a DVE reduce accumulates the chunk sum into pp[g].
                # All-DVE so the scalar (ACT) engine stays a pure DMA queue.
                nm, pp = nm_t[b], pp_t[b]
                if g == 4:
                    nc.vector.tensor_mul(
                        nm[0:1, T : T + 1],
                        pt[4][0:1, 1:2],
                        negrecip[:, T : T + 1],
                    )
                    nc.vector.tensor_copy(pp[0:1, 4:5], nm[0:1, T : T + 1])
                    return
                lo = g * BANK
                nc.vector.tensor_mul(
                    nm[0:1, lo : lo + BANK],
                    pt[g][0:1, 0:BANK],
                    negrecip[:, lo : lo + BANK],
                )
                nc.vector.reduce_sum(
                    pp[0:1, g : g + 1],
                    nm[0:1, lo : lo + BANK],
                    axis=mybir.AxisListType.X,
                )

            qeng = [nc.sync, nc.scalar, nc.gpsimd]

            for b in range(BPC):
                base = b * T * T
                pt = {
                    g: accp.tile(
                        [1, BANK], f32, tag=BANK_TAG[b][g], name=f"p{b}g{g}"
                    )
                    for g in range(5)
                }

                def chunk_dst(c, lo, hi, pt=pt):
                    return pt[c][0:1, lo - c * BANK : hi + 1 - c * BANK]

                # A start=True matmul arms zeroing for its whole 2KB PSUM
                # bank, so every accumulation group begins with one full-bank
                # zeroing matmul; all data matmuls then accumulate.
                for g in range(5):
                    nc.tensor.matmul(
                        pt[g][0:1, 0:BANK],
                        ones[0:2, :],
                        zrhs[:],
                        start=True,
                        stop=False,
                    )

                for q in range(NT // 2):
                    kminU, WU = pinfo[q]
                    i0p = 2 * q * P
                    S2 = data.tile([P, 2 * WU], f32r, tag="S", name="S2")
                    dma_eng = qeng[(b * (NT // 2) + q) % 3]
                    dma_eng.dma_start(
                        S2[:, :],
                        bass.AP(
                            x,
                            base + i0p * (T + 1) + kminU,
                            [[T + 1, P], [P * (T + 1), 2], [1, WU]],
                        ).bitcast(f32r),
                    )
                    for s in range(2):
                        t = 2 * q + s
                        i0, kmin, kmax, kd = tinfo[t]
                        rows = P - 1 if t == NT - 1 else P
                        cb = s * WU - kminU  # + absolute k -> tile column
                        if i0 <= 896:
                            a = 896 - i0
                            nc.vector.tensor_mul(
                                S2[0:rows, cb + a : cb + a + P],
                                S2[0:rows, cb + a : cb + a + P],
                                maskP[0:rows, :],
                            )
                        if i0 >= 896:
                            a = 2944 - i0
                            w = kd + 1 - a
                            nc.vector.tensor_mul(
                                S2[0:rows, cb + a : cb + a + w],
                                S2[0:rows, cb + a : cb + a + w],
                                maskS[0:rows, 0:w],
                            )
                        for c in range(NCH):
                            lo = max(kmin, c * BANK)
                            hi = min(min(kmax, T - 1), c * BANK + BANK - 1)
                            if lo > hi:
                                continue
                            if (hi - lo + 1) % 2 == 1:
                                hi += 1
                                assert hi <= min(kd, c * BANK + BANK - 1)
                            nc.tensor.matmul(
                                chunk_dst(c, lo, hi),
                                ones[0:rows, :],
                                S2[0:rows, cb + lo : cb + hi + 1],
                                start=False,
                                stop=(last[c] == t),
                            )
                        if kmax == T:
                            # fp32r matmul needs N>=2: recompute col 2047 into
                            # a scratch lane and keep only col 2048's sum.
                            nc.tensor.matmul(
                                pt[4][0:1, 0:2],
                                ones[0:rows, :],
                                S2[0:rows, cb + T - 1 : cb + T + 1],
                                start=False,
                                stop=(last[4] == t),
                            )
                        # Interleaved epilogue: read each accumulation group
                        # one tile after it closes, so its bank frees mid-loop
                        # and the next image's matmuls never wait.
                        if t == last[4] + 1:
                            ep_group(b, 4, pt)
                        if t == last[3] + 1:
                            ep_group(b, 3, pt)
                        if b == 1 and t == 2:
                            ep_tail(0, tail_pt)

                if b == 0:
                    tail_pt = pt
                else:
                    ep_tail(1, pt)
    nc.compile()
    return nc


def kernel(**inputs: np.ndarray) -> np.ndarray:
    global _nc_cache
    x = np.asarray(inputs["inputs"], dtype=np.float32)
    assert x.shape == (B, T, T)
    if _nc_cache is None:
        _nc_cache = _build()
    pad = np.zeros(HALF, dtype=np.float32)
    tail = np.zeros(TAILPAD, dtype=np.float32)
    in_maps = [
        {
            "x": np.concatenate(
                [pad, np.ascontiguousarray(x[c * BPC : (c + 1) * BPC]).reshape(-1), tail]
            )
        }
        for c in range(N_CORES)
    ]
    res = run_bass_kernel_spmd(_nc_cache, in_maps, core_ids=list(range(N_CORES)))
    return np.concatenate([r["y"] for r in res.results], axis=0)


# revision 6
# speedup vs baseline: 1.2674x; 1.1358x over previous
"""Trainium2 Bass kernel for nn_DiagMean (histogram_binning).

Computes, per batch image A [T, T]: the mean over each diagonal d = j - i
(for |d| <= T/2, excluding the last element of each diagonal), then centers
across the T+1 diagonal bins and negates.

Strategy (pure data-parallel over batch, 2 images per core on 8 cores):
  - Skewed DMA: for a 128-row tile starting at row i0, the access pattern
    [partition stride T+1, free stride 1] reads S[p, k] = A[i0+p, i0+p+k-T/2],
    so diagonal bin k is a *column* of S. Each partition's read is contiguous
    in HBM, so DMA runs at full per-descriptor rate.
  - PAIRED loads: two consecutive 128-row tiles are fetched by ONE dma_start
    via a 3D access pattern (partition, section, column) into a [128, 2*WU]
    tile, WU = the pair's union column window. Halves the number of DMA
    instructions and semaphore waits; each issue queues ~1.6-2.1 MB of
    descriptors, so the 16 SDMA engines stay fed. Union windows cost ~4%
    extra HBM traffic vs exact per-tile windows.
  - Loads are spread round-robin over THREE issue queues (sync, scalar =
    HWDGE rings; gpsimd = SWDGE) so no single sequencer's sem-reuse window
    (8 DMAHW + 8 DMASW lanes) stalls descriptor supply.
  - Columns fully outside any valid diagonal are never loaded (per-pair
    [kminU, kmaxU] window): ~22% HBM traffic saved vs full rows.
  - The trapezoid validity boundary reduces to two shared 128x128 masks
    (prefix: p+m >= 128, suffix: p+m <= 126) applied to one 128-wide column
    strip per side per tile.
  - Column sums via TensorE: ones[128,1].T @ S chunk -> PSUM [1, N],
    accumulated over the 16 row-tiles of an image (fp32r, 1 cycle/col).

Scheduling:
  - ones/zrhs const DMAs go FIRST on the gpsimd queue so TensorE's zeroing
    matmuls start early.
  - The epilogue is all-DVE (PSUM-read multiply, per-chunk reduce_sum,
    centering); image 0's output store rides the gpsimd queue so it never
    head-of-line blocks a tile-load issue.
  - PSUM bank plan: each of the 5 accumulation groups per image (4 column
    chunks + the k=2048 column) has its own bank; image 1 reuses only banks
    image 0 frees early (c3/c4 are read mid-loop, one tile after their last
    accumulation), so image 1's matmuls never wait on image 0's end.
"""

import numpy as np

import concourse.bacc as bacc
import concourse.bass as bass
import concourse.mybir as mybir
import concourse.tile as tile
from concourse.bass_utils import run_bass_kernel_spmd

B, T = 16, 2048
HALF = T // 2          # 1024
K = T + 1              # 2049 diagonal bins
N_CORES = 8
BPC = B // N_CORES     # images per core
P = 128
NT = T // P            # row tiles per image
BANK = 512             # fp32 elements per PSUM bank
TAILPAD = 2560         # rear pad: pair-DMAs always read 128 rows/section

_nc_cache = None


def _build():
    nc = bacc.Bacc("TRN2", target_bir_lowering=False, debug=False)
    f32 = mybir.dt.float32
    f32r = mybir.dt.float32r

    # Front-padded with HALF zeros so tile 0's skewed read of row 0 lands in
    # the pad instead of out of bounds (the pad zeros are exactly the masked
    # region, so no special-casing). Rear pad covers the last pair's
    # always-128-row section read past the image end (rows there are sliced
    # out of every matmul).
    x = nc.dram_tensor("x", [HALF + BPC * T * T + TAILPAD], f32, kind="ExternalInput")
    y = nc.dram_tensor("y", [BPC, K], f32, kind="ExternalOutput")

    pp_ = np.arange(P)[:, None]
    mm_ = np.arange(P)[None, :]
    maskP_np = (pp_ + mm_ >= P).astype(np.float32)        # prefix validity
    maskS_np = (pp_ + mm_ <= P - 2).astype(np.float32)    # suffix validity
    counts = (T - 1 - np.abs(np.arange(-HALF, HALF + 1))).astype(np.float64)
    # Pad row length to even so SBUF partition strides stay 8-byte aligned.
    KP = K + 1
    negrecip_np = np.zeros((1, KP), dtype=np.float32)
    negrecip_np[0, :K] = (-1.0 / counts).astype(np.float32)
    ones_np = np.ones((P, 1), dtype=np.float32)
    zrhs_np = np.zeros((2, BANK), dtype=np.float32)

    maskP_d = nc.inline_tensor(maskP_np, name="maskP")
    maskS_d = nc.inline_tensor(maskS_np, name="maskS")
    negrecip_d = nc.inline_tensor(negrecip_np, name="negrecip")
    ones_d = nc.inline_tensor(ones_np, name="onesw")
    zrhs_d = nc.inline_tensor(zrhs_np, name="zrhs")

    # Per-tile valid column windows.
    tinfo = []
    for t in range(NT):
        i0 = t * P
        kmin = max(0, 896 - i0)
        kmax = min(T, 3070 - i0)
        kd = kmax + 1 if i0 >= 1024 else kmax  # even-N spare column
        tinfo.append((i0, kmin, kmax, kd))
    NCH = T // BANK                                     # 4 (columns 0..2047)
    last = {}
    for t, (i0, kmin, kmax, kd) in enumerate(tinfo):
        for c in range(NCH):
            lo, hi = c * BANK, c * BANK + BANK - 1
            if kmin <= hi and min(kmax, T - 1) >= lo:
                last[c] = t
        if kmax == T:  # column k=2048 (diag +1024) -> group 4
            last[4] = t

    # Pair (2-tile) DMA windows: union of the two tiles' column windows,
    # width padded even (the pad column is loaded but never consumed).
    pinfo = []
    for q in range(NT // 2):
        t0, t1 = 2 * q, 2 * q + 1
        kminU = min(tinfo[t0][1], tinfo[t1][1])
        kmaxU = max(tinfo[t0][3], tinfo[t1][3])
        WU = kmaxU - kminU + 1
        if WU % 2 == 1:
            WU += 1
        pinfo.append((kminU, WU))

    # PSUM bank plan (16 KB = 8 banks, fully used): per image, bins 0..1535
    # accumulate in ONE 3-bank tile (epilogue reads them with a single DVE
    # mul+reduce), bins 1536..2047 in their own bank (closed and read
    # mid-loop, freeing it for image 1), bin 2048 in a shared bank.
    BANK_TAG = [
        {"m": "bkA", 3: "bk3", 4: "bk7"},
        {"m": "bkB", 3: "bk3", 4: "bk7"},
    ]

    # Load-queue schedule for the 16 pair DMAs (global index b*8+q):
    # sync/scalar are HWDGE rings, gpsimd is SWDGE (slower start, also
    # carries the consts + image-0 store), so gpsimd's input share ends
    # early and the final two pairs stream through both HWDGE rings.
    QSCHED = [0, 1, 2, 0, 1, 2, 0, 1,
              2, 0, 1, 2, 0, 1, 0, 1]

    with tile.TileContext(nc) as tc:
        with (
            tc.tile_pool(name="consts", bufs=1) as consts,
            tc.tile_pool(name="data", bufs=8) as data,
            tc.tile_pool(name="acc", bufs=1, space=bass.MemorySpace.PSUM) as accp,
            tc.tile_pool(name="post", bufs=1) as post,
        ):
            maskP = consts.tile([P, P], f32r)
            maskS = consts.tile([P, P], f32r)
            negrecip = consts.tile([1, KP], f32)
            ones = consts.tile([P, 1], f32r)
            zrhs = consts.tile([2, BANK], f32r)
            # ones/zrhs first: TensorE's zeroing matmuls depend only on these
            # two tiny transfers.
            nc.gpsimd.dma_start(ones[:], ones_d[:].bitcast(f32r))
            nc.gpsimd.dma_start(zrhs[:], zrhs_d[:].bitcast(f32r))
            nc.gpsimd.dma_start(maskP[:], maskP_d[:].bitcast(f32r))
            nc.gpsimd.dma_start(maskS[:], maskS_d[:].bitcast(f32r))
            nc.gpsimd.dma_start(negrecip[:], negrecip_d[:])
            # Collector: touch each const on VectorE once so later DVE ops
            # inherit the const-DMA sync via engine program order instead of
            # each carrying its own semaphore wait (ISA wait-slot limit).
            warm = consts.tile([P, P], f32)
            nc.vector.tensor_copy(warm[:], maskP[:].bitcast(f32))
            nc.vector.tensor_copy(warm[:], maskS[:].bitcast(f32))
            warm2 = consts.tile([1, KP], f32)
            nc.vector.tensor_copy(warm2[:], negrecip[:])

            nm_t = [post.tile([1, KP], f32, tag=f"nm{i}", name=f"nm{i}") for i in range(BPC)]
            s1_t = [post.tile([1, 1], f32, tag=f"s1{i}", name=f"s1{i}") for i in range(BPC)]
            av_t = [post.tile([1, 1], f32, tag=f"av{i}", name=f"av{i}") for i in range(BPC)]
            ot_t = [post.tile([1, KP], f32, tag=f"ot{i}", name=f"ot{i}") for i in range(BPC)]
            pp_t = [post.tile([1, 6], f32, tag=f"pp{i}", name=f"pp{i}") for i in range(BPC)]

            def ep_tail(b, pt):
                # Merged-bank read (bins 0..1535 in one mul+reduce), combine,
                # centering, output store. For image 0 this is issued a few
                # tiles INTO image 1's loop.
                nm, s1, avgneg, ot = nm_t[b], s1_t[b], av_t[b], ot_t[b]
                nc.vector.tensor_mul(
                    nm[0:1, 0 : 3 * BANK],
                    pt["m"][0:1, 0 : 3 * BANK],
                    negrecip[:, 0 : 3 * BANK],
                )
                nc.vector.reduce_sum(
                    pp_t[b][0:1, 0:1],
                    nm[0:1, 0 : 3 * BANK],
                    axis=mybir.AxisListType.X,
                )
                nc.vector.reduce_sum(
                    s1[0:1, 0:1], pp_t[b][0:1, 0:3], axis=mybir.AxisListType.X
                )
                nc.vector.tensor_scalar_mul(
                    avgneg[0:1, 0:1], s1[0:1, 0:1], -1.0 / K
                )
                nc.vector.tensor_scalar_add(
                    ot[0:1, 0:K], nm[0:1, 0:K], avgneg[0:1, 0:1]
                )
                if b == 0:
                    # keeps the HWDGE queues' tile loads unblocked.
                    nc.gpsimd.dma_start(y[b : b + 1, :], ot[0:1, 0:K])
                else:
                    nc.sync.dma_start(y[b : b + 1, :], ot[0:1, 0:K])

            def ep_group(b, g, pt):
                # negmean for group g (the DVE read frees its PSUM bank),
                # then a DVE reduce accumulates the chunk sum into pp.
                # All-DVE so the scalar (ACT) engine stays a pure DMA queue.
                nm, pp = nm_t[b], pp_t[b]
                if g == 4:
                    nc.vector.tensor_mul(
                        nm[0:1, T : T + 1],
                        pt[4][0:1, 1:2],
                        negrecip[:, T : T + 1],
                    )
                    nc.vector.tensor_copy(pp[0:1, 2:3], nm[0:1, T : T + 1])
                    return
                lo = g * BANK
                nc.vector.tensor_mul(
                    nm[0:1, lo : lo + BANK],
                    pt[g][0:1, 0:BANK],
                    negrecip[:, lo : lo + BANK],
                )
                nc.vector.reduce_sum(
                    pp[0:1, 1:2],
                    nm[0:1, lo : lo + BANK],
                    axis=mybir.AxisListType.X,
                )

            qeng = [nc.sync, nc.scalar, nc.gpsimd]

            for b in range(BPC):
                base = b * T * T
                pt = {
                    "m": accp.tile(
                        [1, 3 * BANK], f32, tag=BANK_TAG[b]["m"], name=f"p{b}m"
                    ),
                    3: accp.tile([1, BANK], f32, tag=BANK_TAG[b][3], name=f"p{b}g3"),
                    4: accp.tile([1, BANK], f32, tag=BANK_TAG[b][4], name=f"p{b}g4"),
                }

                def chunk_dst(c, lo, hi, pt=pt):
                    if c < 3:
                        return pt["m"][0:1, lo : hi + 1]
                    return pt[3][0:1, lo - 3 * BANK : hi + 1 - 3 * BANK]

                # A start=True matmul arms zeroing for its whole 2KB PSUM
                # bank, so every accumulation group begins with one full-bank
                # zeroing matmul; all data matmuls then accumulate.
                for c in range(3):
                    nc.tensor.matmul(
                        pt["m"][0:1, c * BANK : (c + 1) * BANK],
                        ones[0:2, :],
                        zrhs[:],
                        start=True,
                        stop=False,
                    )
                for g in (3, 4):
                    nc.tensor.matmul(
                        pt[g][0:1, 0:BANK],
                        ones[0:2, :],
                        zrhs[:],
                        start=True,
                        stop=False,
                    )

                for q in range(NT // 2):
                    kminU, WU = pinfo[q]
                    i0p = 2 * q * P
                    S2 = data.tile([P, 2 * WU], f32r, tag="S", name="S2")
                    dma_eng = qeng[QSCHED[b * (NT // 2) + q]]
                    dma_eng.dma_start(
                        S2[:, :],
                        bass.AP(
                            x,
                            base + i0p * (T + 1) + kminU,
                            [[T + 1, P], [P * (T + 1), 2], [1, WU]],
                        ).bitcast(f32r),
                    )
                    for s in range(2):
                        t = 2 * q + s
                        i0, kmin, kmax, kd = tinfo[t]
                        rows = P - 1 if t == NT - 1 else P
                        cb = s * WU - kminU  # + absolute k -> tile column
                        if i0 <= 896:
                            a = 896 - i0
                            nc.vector.tensor_mul(
                                S2[0:rows, cb + a : cb + a + P],
                                S2[0:rows, cb + a : cb + a + P],
                                maskP[0:rows, :],
                            )
                        if i0 >= 896:
                            a = 2944 - i0
                            w = kd + 1 - a
                            nc.vector.tensor_mul(
                                S2[0:rows, cb + a : cb + a + w],
                                S2[0:rows, cb + a : cb + a + w],
                                maskS[0:rows, 0:w],
                            )
                        for c in range(NCH):
                            lo = max(kmin, c * BANK)
                            hi = min(min(kmax, T - 1), c * BANK + BANK - 1)
                            if lo > hi:
                                continue
                            if (hi - lo + 1) % 2 == 1:
                                hi += 1
                                assert hi <= min(kd, c * BANK + BANK - 1)
                            nc.tensor.matmul(
                                chunk_dst(c, lo, hi),
                                ones[0:rows, :],
                                S2[0:rows, cb + lo : cb + hi + 1],
                                start=False,
                                stop=(last[c] == t),
                            )
                        if kmax == T:
                            # fp32r matmul needs N>=2: recompute col 2047 into
                            # a scratch lane and keep only col 2048's sum.
                            nc.tensor.matmul(
                                pt[4][0:1, 0:2],
                                ones[0:rows, :],
                                S2[0:rows, cb + T - 1 : cb + T + 1],
                                start=False,
                                stop=(last[4] == t),
                            )
                        # Interleaved epilogue: read each accumulation group
                        # one tile after it closes, so its bank frees mid-loop
                        # and the next image's matmuls never wait.
                        if t == last[4] + 1:
                            ep_group(b, 4, pt)
                        if t == last[3] + 1:
                            ep_group(b, 3, pt)
                        if b == 1 and t == 2:
                            ep_tail(0, tail_pt)

                if b == 0:
                    tail_pt = pt
                else:
                    ep_tail(1, pt)
    nc.compile()
    return nc


def kernel(**inputs: np.ndarray) -> np.ndarray:
    global _nc_cache
    x = np.asarray(inputs["inputs"], dtype=np.float32)
    assert x.shape == (B, T, T)
    if _nc_cache is None:
        _nc_cache = _build()
    pad = np.zeros(HALF, dtype=np.float32)
    tail = np.zeros(TAILPAD, dtype=np.float32)
    in_maps = [
        {
            "x": np.concatenate(
                [pad, np.ascontiguousarray(x[c * BPC : (c + 1) * BPC]).reshape(-1), tail]
            )
        }
        for c in range(N_CORES)
    ]
    res = run_bass_kernel_spmd(_nc_cache, in_maps, core_ids=list(range(N_CORES)))
    return np.concatenate([r["y"] for r in res.results], axis=0)
